# revision 23
# baseline (speedup 1.0000x reference)
"""Trainium2 Bass kernel for nn_Attn: per-sample neighbor attention softmax.

Math: reference computes
    temp[b]   = encoder_outputs[b, current_index]              # [64]
    energy    = enc_nb @ W.T + bias                            # [B, N, 64]
    logits    = einsum('bnd,bd->bn', energy, temp)             # [B, N]
    out       = softmax(logits, axis=1)

Algebraic rewrite used here:
    logits[b, n] = enc_nb[b, n] . (W.T @ temp[b]) + bias . temp[b]
The bias term is constant over n for a given sample, so it cancels in the
softmax; we drop it. What remains is a per-sample matvec against
v[b] = W.T @ temp[b], which makes the kernel HBM-bound (the 537 MB
enc_nb stream), not matmul-bound.

Device layout (per core, 16 samples):
    partition p = b*8 + j   (b in [0,16): sample, j in [0,8): n-octant)
    tile column t in [0,256): n = j*256 + t
    v_rep [128, 512]: row p holds v[b]  (built by one PE matmul from a
        host-prepped temp.T with each column replicated 8x)
    per tile: DVE fused multiply+row-sum (scalar_tensor_tensor with
        accum_out) -> scores[:, t]
    softmax: a DRAM bounce regroups scores [128,256] -> [16,2048] so the
        whole softmax runs on free-dim ops (no cross-partition reduction).
"""

from contextlib import ExitStack

import numpy as np

import concourse.bacc as bacc
import concourse.bass as bass
import concourse.mybir as mybir
import concourse.tile as tile
from concourse.bass_utils import run_bass_kernel_spmd

N_CORES = 8
B = 128          # batch
N = 2048         # neighbors per sample
S0 = 512         # neighbor feature dim
D = 64           # query feature dim
BC = B // N_CORES  # samples per core = 16
J = 8            # n-octants per sample -> BC * J = 128 partitions
TT = N // J      # tile columns = 256
G = 8            # tile columns loaded per DMA (2 MB per dma_start)
LOAD_BUFS = 8    # load-pool depth (DMA pipelining)
FP32 = mybir.dt.float32


def _emit(ctx: ExitStack, tc: "tile.TileContext", enc, wt, out):
    nc = tc.nc
    const_pool = ctx.enter_context(tc.tile_pool(name="const", bufs=1))
    load_pool = ctx.enter_context(tc.tile_pool(name="load", bufs=LOAD_BUFS))
    scratch_pool = ctx.enter_context(tc.tile_pool(name="scratch", bufs=6))
    psum_pool = ctx.enter_context(tc.tile_pool(name="psum", bufs=1, space="PSUM"))
    sm_pool = ctx.enter_context(tc.tile_pool(name="sm", bufs=1))

    # v_rep[p, s] = sum_d tt_rep[d, p] * W[d, s]  (tt_rep replicates each
    # sample's temp across its 8 partitions, so one matmul builds the
    # broadcast operand directly). W and tt_rep arrive packed in one
    # [64, 640] input: the PE LoadWeights slot only tolerates a single
    # sync wait, so the matmul must depend on exactly one DMA.
    wt_sb = const_pool.tile([D, S0 + BC * J], FP32)
    nc.sync.dma_start(wt_sb[:], wt[:])
    vrep_ps = psum_pool.tile([BC * J, S0], FP32)
    nc.tensor.matmul(vrep_ps[:], wt_sb[:, S0:], wt_sb[:, :S0])
    # Copy on the vector engine: the TTRs below read vrep from the same
    # engine, so this dependency stays program-order and never becomes a
    # sync wait (the TTR ISA struct tolerates only one wait — the DMA's).
    vrep = const_pool.tile([BC * J, S0], FP32)
    nc.vector.tensor_copy(vrep[:], vrep_ps[:])

    scores = const_pool.tile([BC * J, TT], FP32)

    # [16, 2048, 512] -> [(b j)=128, t=256, s=512]; consecutive t are
    # consecutive DRAM rows, so each partition reads G*2KB contiguous.
    enc_r = enc.rearrange("b (j t) s -> (b j) t s", j=J)

    for g in range(TT // G):
        et = load_pool.tile([BC * J, G, S0], FP32)
        nc.sync.dma_start(et[:], enc_r[:, g * G : (g + 1) * G, :])
        for k in range(G):
            c = g * G + k
            # scalar_tensor_tensor lowers to native InstTensorScalarPtr
            # (tensor_tensor_reduce is raw InstISA, whose sync struct
            # tolerates only one wait — too few when a load's DMA spans
            # several queues): out = (in0*1)*in1, accum_out = sum(out).
            prod = scratch_pool.tile([BC * J, S0], FP32, tag="prod_dve")
            nc.vector.scalar_tensor_tensor(
                out=prod[:],
                in0=et[:, k, :],
                scalar=1.0,
                in1=vrep[:],
                op0=mybir.AluOpType.mult,
                op1=mybir.AluOpType.mult,
                accum_out=scores[:, c : c + 1],
            )

    # Regroup so each sample's 2048 scores live on one partition's free dim.
    # SBUF APs can't move partition sub-dims into free dims, so bounce
    # through DRAM: [128, 256] written contiguously IS [16, 2048] in
    # (b, j, t) order when read back flat.
    dram_pool = ctx.enter_context(tc.tile_pool(name="dram", bufs=1, space="DRAM"))
    scores_dram = dram_pool.tile([BC * J, TT], FP32)
    nc.sync.dma_start(scores_dram[:], scores[:])
    scores_r = sm_pool.tile([BC, N], FP32)
    nc.sync.dma_start(scores_r[:], scores_dram[:].rearrange("(b j) t -> b (j t)", j=J))

    nmax = sm_pool.tile([BC, 1], FP32)
    nc.vector.tensor_reduce(
        out=nmax[:],
        in_=scores_r[:],
        axis=mybir.AxisListType.X,
        op=mybir.AluOpType.max,
        negate=True,
    )
    probs = sm_pool.tile([BC, N], FP32)
    sumexp = sm_pool.tile([BC, 1], FP32)
    nc.scalar.activation(
        out=probs[:],
        in_=scores_r[:],
        func=mybir.ActivationFunctionType.Exp,
        bias=nmax[:],
        scale=1.0,
        accum_out=sumexp[:],
    )
    recip = sm_pool.tile([BC, 1], FP32)
    nc.vector.reciprocal(recip[:], sumexp[:])
    probs2 = sm_pool.tile([BC, N], FP32)
    nc.scalar.activation(
        out=probs2[:],
        in_=probs[:],
        func=mybir.ActivationFunctionType.Copy,
        scale=recip[:],
    )
    nc.sync.dma_start(out[:], probs2[:])


_NC_CACHE = {}


def build_bass(reps: int = 1) -> bass.Bass:
    """reps>1 emits the body that many times in one NEFF (used by the
    timing harness to cancel per-dispatch overhead)."""
    if reps in _NC_CACHE:
        return _NC_CACHE[reps]
    # Bacc (not raw Bass): its compile() splits multi-sem waits into event
    # semaphores (TRN2 allows one wait per instruction), moves matmul waits
    # to ldweights, and populates extended-ISA instruction bytes.
    nc = bacc.Bacc(trn_type="TRN2", target_bir_lowering=False, debug=False)
    enc = nc.dram_tensor("enc", [BC, N, S0], FP32, kind="ExternalInput").ap()
    wt = nc.dram_tensor("wt", [D, S0 + BC * J], FP32, kind="ExternalInput").ap()
    out = nc.dram_tensor("out", [BC, N], FP32, kind="ExternalOutput").ap()
    with tile.TileContext(nc) as tc:
        for _ in range(reps):
            with ExitStack() as ctx:
                _emit(ctx, tc, enc, wt, out)
    nc.compile()
    _NC_CACHE[reps] = nc
    return nc


def make_in_maps(inputs: dict) -> list[dict]:
    enc_out = np.ascontiguousarray(np.asarray(inputs["encoder_outputs"], dtype=np.float32))
    enc_nb = np.asarray(inputs["encoder_outputs_neighbor"], dtype=np.float32)
    w = np.ascontiguousarray(np.asarray(inputs["W"], dtype=np.float32))
    idx = int(np.asarray(inputs["current_index"]))
    temp = enc_out[:, idx, :]  # [B, D]

    in_maps = []
    for c in range(N_CORES):
        tb = temp[c * BC : (c + 1) * BC]  # [16, 64]
        # tt_rep[d, b*8+j] = tb[b, d]; packed after W into one [64, 640] input
        tt_rep = np.repeat(tb, J, axis=0).T  # [64, 128]
        wt = np.ascontiguousarray(np.concatenate([w, tt_rep], axis=1))
        in_maps.append(
            {
                "enc": np.ascontiguousarray(enc_nb[c * BC : (c + 1) * BC]),
                "wt": wt,
            }
        )
    return in_maps


def kernel(**inputs) -> np.ndarray:
    nc = build_bass()
    in_maps = make_in_maps(inputs)
    res = run_bass_kernel_spmd(nc, in_maps, core_ids=list(range(N_CORES)))
    return np.concatenate([res.results[c]["out"] for c in range(N_CORES)], axis=0)
